# revision 6
# baseline (speedup 1.0000x reference)
"""Trainium2 Bass kernel for nn_ConvblockWithTarget (dense_cnn).

Reference computation (B=4, L=32768, C=64, K=7, T=16378):
  - unfold x into windows of 2K at stride 2 -> xp[b,c,t,j] = x[b, 2t+j, c]
  - dynamic gates: wfull[b,d,l,m] = sum_c x[b, 12+l, c] * weights[d,c,m]
    gate[b,d,t,j] = wfull[b,d,2t+(j%2), j//2];  g = tanh(gate)
  - y[b,t,d] = sum_j xp * g  + skip x[b, 12+2t, d]
  - y_ = batchnorm1(y);  z = y_ @ w_conv; z = gelu_tanh(batchnorm2(z)); out = y_ + z

Sharding: 8 cores = (batch b, sequence-half h).  Each core computes
P = T//2 output positions for one batch.  Batch-norm statistics are
global -> two tiny AllReduces of per-channel (sum, sumsq).

On-chip layout is channel-major ("transposed"): SBUF partitions hold
(channel d, phase/block parity).  Host-side prep uploads x pre-split
into even/odd phases, transposed and cast to fp16:
  XB  = [xE | xO] stacked on partitions (rows 0-63 = even-phase, 64-127 odd)
  XB2 = [xO | xE] (for the second pair of concurrent PE quadrants)
  XS1 = XB shifted by one element (keeps DVE 2x mode for odd window taps)
"""

import os
import numpy as np

K = 7


# ----------------------------------------------------------------------------
# device program
# ----------------------------------------------------------------------------

def _build_program(NSB, NTOT, P, n_cores=8):
    """Emit the SPMD Bass program. NSB = superblocks (1024 outputs each) per
    core; NTOT = total valid samples across all cores (= B*T); P = valid
    outputs per core."""
    from contextlib import ExitStack

    import concourse.bacc as bacc
    import concourse.bass as bass
    import concourse.mybir as mybir
    import concourse.tile as tile
    from concourse.alu_op_type import AluOpType as alu

    dt = mybir.dt
    f32, f16 = dt.float32, dt.float16
    AF = mybir.ActivationFunctionType
    EPS = 1e-5
    TLOC = 1024 * NSB + 16
    FO = 512 * NSB  # output free size per partition

    nc = bacc.Bacc("TRN2", target_bir_lowering=False, debug=False,
                   num_devices=n_cores)

    xb_d = nc.dram_tensor("xb", [128, TLOC], f16, kind="ExternalInput")
    xb2_d = nc.dram_tensor("xb2", [128, TLOC], f16, kind="ExternalInput")
    xs1_d = nc.dram_tensor("xs1", [128, TLOC], f16, kind="ExternalInput")
    w2_d = nc.dram_tensor("w2", [128, 64 * K], f16, kind="ExternalInput")
    sid_d = nc.dram_tensor("sid", [128, 128], f16, kind="ExternalInput")
    wc_d = nc.dram_tensor("wc", [128, 64], f16, kind="ExternalInput")
    cst_d = nc.dram_tensor("cst", [128, 8], f32, kind="ExternalInput")
    out_d = nc.dram_tensor("out", [128, FO], f16, kind="ExternalOutput")

    groups = [list(range(n_cores))]

    with ExitStack() as es:
        tc = es.enter_context(tile.TileContext(nc))
        cp = es.enter_context(tc.tile_pool(name="const", bufs=1))
        dp = es.enter_context(tc.tile_pool(name="dram", bufs=1, space="DRAM"))
        gsb = es.enter_context(tc.tile_pool(name="gsb", bufs=3))
        prsb = es.enter_context(tc.tile_pool(name="prsb", bufs=3))
        osb = es.enter_context(tc.tile_pool(name="osb", bufs=2))
        scp = es.enter_context(tc.tile_pool(name="scp", bufs=1))

        # persistent SBUF
        xb_sb = cp.tile([128, TLOC], f16)
        xb2_sb = cp.tile([128, TLOC], f16)
        xs1_sb = cp.tile([128, TLOC], f16)
        w2_sb = cp.tile([128, 64 * K], f16)
        sid_sb = cp.tile([128, 128], f16)
        wc_sb = cp.tile([128, 64], f16)
        cst_sb = cp.tile([128, 8], f32)
        y_sb = cp.tile([128, FO], f16)
        yn_sb = cp.tile([128, FO], f16)
        yst = cp.tile([128, 6 * NSB], f32)
        zst = cp.tile([128, 6 * NSB], f32)

        nc.sync.dma_start(out=xb_sb[:], in_=xb_d.ap())
        nc.sync.dma_start(out=xb2_sb[:], in_=xb2_d.ap())
        nc.sync.dma_start(out=xs1_sb[:], in_=xs1_d.ap())
        nc.sync.dma_start(out=w2_sb[:], in_=w2_d.ap())
        nc.sync.dma_start(out=sid_sb[:], in_=sid_d.ap())
        nc.sync.dma_start(out=wc_sb[:], in_=wc_d.ap())
        nc.sync.dma_start(out=cst_sb[:], in_=cst_d.ap())

        SI = sid_sb[:, 0:64]     # [I64; I64]
        SIE = sid_sb[:, 64:128]  # [I64; 0]

        def mm(out, lhsT, rhs, tp, start=True, stop=True):
            nc.tensor.matmul(out, lhsT, rhs, start=start, stop=stop,
                             tile_position=tp, skip_group_check=True)

        def allreduce_2(arv, tagn):
            """AllReduce a [128,2] f32 (sum, sumsq); returns [128,2] totals
            combined across cores AND across the two partition halves
            (channel d and d+64 hold the same channel)."""
            arin = dp.tile([128, 2], f32, name=f"arin{tagn}")
            arout = dp.tile([128, 2], f32, name=f"arout{tagn}",
                            addr_space="Shared")
            nc.sync.dma_start(out=arin[:], in_=arv[:])
            nc.gpsimd.collective_compute(
                "AllReduce", alu.add, replica_groups=groups,
                ins=[arin.opt()], outs=[arout.opt()])
            gat = scp.tile([128, 4], f32, name=f"gat{tagn}")
            src = arout.rearrange("(a d) s -> d a s", a=2)
            nc.sync.dma_start(
                out=gat[0:64, :].rearrange("d (a s) -> d a s", a=2), in_=src)
            nc.sync.dma_start(
                out=gat[64:128, :].rearrange("d (a s) -> d a s", a=2), in_=src)
            tot = scp.tile([128, 2], f32, name=f"tot{tagn}")
            nc.vector.tensor_tensor(tot[:], gat[:, 0:2], gat[:, 2:4], alu.add)
            return tot

        def rsqrt_of(var_ap, tagn):
            """[128,1] rsqrt(var + EPS) via ACT sqrt + DVE recip + one Newton
            step (r1 = r0*(1.5 - 0.5*v*r0^2))."""
            vp = scp.tile([128, 1], f32, name=f"vp{tagn}")
            nc.vector.tensor_scalar(vp[:], var_ap, EPS, None, alu.add)
            sq = scp.tile([128, 1], f32, name=f"sq{tagn}")
            nc.scalar.sqrt(sq[:], vp[:])
            r0 = scp.tile([128, 1], f32, name=f"r0{tagn}")
            nc.vector.reciprocal(r0[:], sq[:])
            t2 = scp.tile([128, 1], f32, name=f"t2{tagn}")
            nc.vector.tensor_tensor(t2[:], r0[:], r0[:], alu.mult)
            nc.vector.tensor_tensor(t2[:], t2[:], vp[:], alu.mult)
            nc.vector.tensor_scalar(t2[:], t2[:], -0.5, 1.5, alu.mult, alu.add)
            rs = scp.tile([128, 1], f32, name=f"rs{tagn}")
            nc.vector.tensor_tensor(rs[:], r0[:], t2[:], alu.mult)
            return rs

        def sums_from_bnstats(stt, tagn):
            """bn_stats slots [128, 6*NSB] -> [128,2] (sum, sumsq) local."""
            agg = scp.tile([128, 2], f32, name=f"agg{tagn}")
            nc.vector.bn_aggr(agg[:], stt[:])
            cnt = float(512 * NSB)
            arv = scp.tile([128, 2], f32, name=f"arv{tagn}")
            nc.vector.tensor_scalar(arv[:, 0:1], agg[:, 0:1], cnt, None,
                                    alu.mult)
            m2 = scp.tile([128, 1], f32, name=f"m2{tagn}")
            nc.vector.tensor_tensor(m2[:], agg[:, 0:1], agg[:, 0:1], alu.mult)
            nc.vector.tensor_tensor(m2[:], m2[:], agg[:, 1:2], alu.add)
            nc.vector.tensor_scalar(arv[:, 1:2], m2[:], cnt, None, alu.mult)
            return arv

        # ------------------------------------------------------------------
        # Phase A: gates -> tanh -> windowed products -> PE-summed y
        # ------------------------------------------------------------------
        with tc.tile_pool(name="psA", bufs=2, space="PSUM") as psA:
            for sb in range(NSB):
                b6 = 1024 * sb + 6
                y_ps = psA.tile([128, 512], f32, tag="y_ps", name=f"y_ps{sb}")
                # skip connection seeds the accumulation (lhsT [I64;0])
                mm(y_ps[0:64, :], SIE, xb_sb[:, b6:b6 + 512], (0, 0),
                   start=True, stop=False)
                mm(y_ps[64:128, :], SIE, xb_sb[:, b6 + 512:b6 + 1024], (0, 64),
                   start=True, stop=False)
                for m in range(K):
                    wsl = slice(64 * m, 64 * m + 64)
                    g_ps = psA.tile([128, 1024], f32, tag="g_ps",
                                    name=f"g_ps{sb}_{m}")
                    # 4 concurrent quadrant matmuls (rows=contraction c,
                    # cols=output partitions)
                    mm(g_ps[0:64, 0:512], w2_sb[0:64, wsl],
                       xb_sb[0:64, b6:b6 + 512], (0, 0))
                    mm(g_ps[64:128, 0:512], w2_sb[64:128, wsl],
                       xb_sb[64:128, b6:b6 + 512], (64, 64))
                    mm(g_ps[64:128, 512:1024], w2_sb[0:64, wsl],
                       xb2_sb[0:64, b6 + 512:b6 + 1024], (0, 64))
                    mm(g_ps[0:64, 512:1024], w2_sb[64:128, wsl],
                       xb2_sb[64:128, b6 + 512:b6 + 1024], (64, 0))
                    gs = gsb.tile([128, 1024], f16, tag="gs",
                                  name=f"gs{sb}_{m}")
                    nc.scalar.activation(gs[:], g_ps[:], AF.Tanh)
                    pr = prsb.tile([128, 1024], f16, tag="pr",
                                   name=f"pr{sb}_{m}")
                    if m % 2 == 0:
                        xop = xb_sb[:, 1024 * sb + m:1024 * sb + m + 1024]
                    else:
                        xop = xs1_sb[:, 1024 * sb + m - 1:
                                     1024 * sb + m - 1 + 1024]
                    nc.vector.tensor_tensor(pr[:], gs[:], xop, alu.mult)
                    mm(y_ps[0:64, :], SI, pr[:, 0:512], (0, 0),
                       start=False, stop=(m == K - 1))
                    mm(y_ps[64:128, :], SI, pr[:, 512:1024], (0, 64),
                       start=False, stop=(m == K - 1))
                # zero positions beyond this core's valid range (i >= P):
                # they hold the neighbouring half's data and would corrupt
                # the global statistics.
                jA = max(0, min(512, P - 1024 * sb))
                jB = max(0, min(512, P - 1024 * sb - 512))
                if jA < 512:
                    nc.vector.tensor_scalar(y_ps[0:64, jA:512],
                                            y_ps[0:64, jA:512], 0.0, None,
                                            alu.mult)
                if jB < 512:
                    nc.vector.tensor_scalar(y_ps[64:128, jB:512],
                                            y_ps[64:128, jB:512], 0.0, None,
                                            alu.mult)
                nc.vector.bn_stats(yst[:, 6 * sb:6 * sb + 6], y_ps[:])
                nc.vector.tensor_copy(y_sb[:, 512 * sb:512 * sb + 512],
                                      y_ps[:])

            # ---- AllReduce 1: y statistics ----
            arv1 = sums_from_bnstats(yst, 1)
            tot1 = allreduce_2(arv1, 1)

            mean1 = scp.tile([128, 1], f32)
            nc.vector.tensor_scalar(mean1[:], tot1[:, 0:1], 1.0 / NTOT, None,
                                    alu.mult)
            var1 = scp.tile([128, 1], f32)
            nc.vector.tensor_scalar(var1[:], tot1[:, 1:2], 1.0 / NTOT, None,
                                    alu.mult)
            msq = scp.tile([128, 1], f32)
            nc.vector.tensor_tensor(msq[:], mean1[:], mean1[:], alu.mult)
            nc.vector.tensor_sub(var1[:], var1[:], msq[:])
            rstd1 = rsqrt_of(var1[:], 1)
            s1h = scp.tile([128, 1], f32)
            nc.vector.tensor_tensor(s1h[:], rstd1[:], cst_sb[:, 0:1], alu.mult)
            t1 = scp.tile([128, 1], f32)
            nc.vector.tensor_tensor(t1[:], mean1[:], s1h[:], alu.mult)
            nc.vector.tensor_sub(t1[:], cst_sb[:, 1:2], t1[:])
            # conv weights pre-scaled by s1h (per input channel = partition)
            wcs = scp.tile([128, 64], f16)
            nc.vector.tensor_scalar(wcs[:], wc_sb[:], s1h[:], None, alu.mult)
            t1h = scp.tile([128, 1], f16)
            nc.vector.tensor_copy(t1h[:], t1[:])
            # c_row[d'] = sum_d t1[d] * wc[d,d']  (borrow a y_ps slot)
            c_ps = psA.tile([128, 1], f32, tag="y_ps")
            mm(c_ps[0:64, :], wc_sb[0:64, :], t1h[0:64, :], (0, 0))
            mm(c_ps[64:128, :], wc_sb[64:128, :], t1h[64:128, :], (64, 64))
            c_row = scp.tile([128, 1], f32)
            nc.vector.tensor_copy(c_row[:], c_ps[:])

        # BN1 application (overlaps collective 2)
        for sb in range(NSB):
            osl = slice(512 * sb, 512 * sb + 512)
            nc.vector.tensor_scalar(yn_sb[:, osl], y_sb[:, osl], s1h[:],
                                    t1[:], alu.mult, alu.add)

        # ------------------------------------------------------------------
        # Phase B: 1x1 conv (on raw y; scale folded in weights) + z stats
        # ------------------------------------------------------------------
        with tc.tile_pool(name="psZ", bufs=NSB, space="PSUM") as psZ:
            z_tiles = []
            for sb in range(NSB):
                osl = slice(512 * sb, 512 * sb + 512)
                z_ps = psZ.tile([128, 512], f32, tag="z_ps", name=f"z_ps{sb}")
                mm(z_ps[0:64, :], wcs[0:64, :], y_sb[0:64, osl], (0, 0))
                mm(z_ps[64:128, :], wcs[64:128, :], y_sb[64:128, osl],
                   (64, 64))
                nc.vector.bn_stats(zst[:, 6 * sb:6 * sb + 6], z_ps[:])
                z_tiles.append(z_ps)

            # ---- AllReduce 2: z statistics (of z_mm; z = z_mm + c_row) ----
            arv2 = sums_from_bnstats(zst, 2)
            tot2 = allreduce_2(arv2, 2)

            szm = tot2[:, 0:1]   # sum z_mm
            sz2m = tot2[:, 1:2]  # sum z_mm^2
            ncr = scp.tile([128, 1], f32)
            nc.vector.tensor_scalar(ncr[:], c_row[:], float(NTOT), None,
                                    alu.mult)
            sz = scp.tile([128, 1], f32)
            nc.vector.tensor_tensor(sz[:], szm, ncr[:], alu.add)
            # sumsq z = sumsq z_mm + 2*c_row*sum z_mm + NTOT*c_row^2
            a1 = scp.tile([128, 1], f32)
            nc.vector.tensor_tensor(a1[:], c_row[:], szm, alu.mult)
            nc.vector.tensor_scalar(a1[:], a1[:], 2.0, None, alu.mult)
            a2 = scp.tile([128, 1], f32)
            nc.vector.tensor_tensor(a2[:], c_row[:], ncr[:], alu.mult)
            sz2 = scp.tile([128, 1], f32)
            nc.vector.tensor_tensor(sz2[:], sz2m, a1[:], alu.add)
            nc.vector.tensor_tensor(sz2[:], sz2[:], a2[:], alu.add)
            mean2 = scp.tile([128, 1], f32)
            nc.vector.tensor_scalar(mean2[:], sz[:], 1.0 / NTOT, None,
                                    alu.mult)
            var2 = scp.tile([128, 1], f32)
            nc.vector.tensor_scalar(var2[:], sz2[:], 1.0 / NTOT, None,
                                    alu.mult)
            msq2 = scp.tile([128, 1], f32)
            nc.vector.tensor_tensor(msq2[:], mean2[:], mean2[:], alu.mult)
            nc.vector.tensor_sub(var2[:], var2[:], msq2[:])
            rstd2 = rsqrt_of(var2[:], 2)
            s2h = scp.tile([128, 1], f32)
            nc.vector.tensor_tensor(s2h[:], rstd2[:], cst_sb[:, 2:3], alu.mult)
            # gelu input = z_mm*s2h + gb, gb = b2 + s2h*(c_row - mean2)
            gb = scp.tile([128, 1], f32)
            nc.vector.tensor_sub(gb[:], c_row[:], mean2[:])
            nc.vector.tensor_tensor(gb[:], gb[:], s2h[:], alu.mult)
            nc.vector.tensor_tensor(gb[:], gb[:], cst_sb[:, 3:4], alu.add)

            # --------------------------------------------------------------
            # Phase C: gelu + residual + store
            # --------------------------------------------------------------
            for sb in range(NSB):
                osl = slice(512 * sb, 512 * sb + 512)
                ge = gsb.tile([128, 512], f16, tag="ge", name=f"ge{sb}")
                nc.scalar.activation(ge[:], z_tiles[sb][:],
                                     AF.Gelu_apprx_tanh, bias=gb[:],
                                     scale=s2h[:])
                of = osb.tile([128, 512], f16, tag="of", name=f"of{sb}")
                nc.vector.tensor_tensor(of[:], ge[:], yn_sb[:, osl], alu.add)
                nc.sync.dma_start(out=out_d.ap()[:, osl], in_=of[:])

    nc.compile()
    return nc


# ----------------------------------------------------------------------------
# host side
# ----------------------------------------------------------------------------

_CACHE = {}


def _get_program(NSB, NTOT, P):
    key = (NSB, NTOT, P)
    if key not in _CACHE:
        _CACHE[key] = _build_program(NSB, NTOT, P)
    return _CACHE[key]


def _prep_inputs(x, weights, w_conv, scale1, bias1, scale2, bias2):
    """Host-side layout prep. Returns (in_maps, meta)."""
    B, L, C = x.shape
    T = (L - 2 * K) // 2 + 1
    assert T % 2 == 0
    P = T // 2
    NSB = -(-P // 1024)
    TLOC = 1024 * NSB + 16
    NTOT = B * T
    LH = L // 2

    x = np.asarray(x, np.float32)
    xE = np.ascontiguousarray(x[:, 0::2, :].transpose(0, 2, 1)).astype(np.float16)
    xO = np.ascontiguousarray(x[:, 1::2, :].transpose(0, 2, 1)).astype(np.float16)

    w2 = np.zeros((128, 64 * K), np.float16)
    wt = np.asarray(weights, np.float32)  # (C,C,K) = (d,c,m)
    for m in range(K):
        w2[0:64, 64 * m:64 * m + 64] = wt[:, :, m].T.astype(np.float16)
    w2[64:128] = w2[0:64]

    sid = np.zeros((128, 128), np.float16)
    eye = np.eye(64, dtype=np.float16)
    sid[0:64, 0:64] = eye
    sid[64:128, 0:64] = eye
    sid[0:64, 64:128] = eye

    wc = np.zeros((128, 64), np.float16)
    wc[0:64] = np.asarray(w_conv, np.float16)
    wc[64:128] = wc[0:64]

    cst = np.zeros((128, 8), np.float32)
    for i, v in enumerate([scale1, bias1, scale2, bias2]):
        v = np.asarray(v, np.float32)
        cst[0:64, i] = v
        cst[64:128, i] = v

    in_maps = []
    for core in range(2 * B):
        b, h = core // 2, core % 2
        i0 = h * P
        n = min(TLOC, LH - i0)
        xbc = np.zeros((128, TLOC), np.float16)
        xbc[0:64, :n] = xE[b, :, i0:i0 + n]
        xbc[64:128, :n] = xO[b, :, i0:i0 + n]
        xb2c = np.zeros((128, TLOC), np.float16)
        xb2c[0:64, :n] = xO[b, :, i0:i0 + n]
        xb2c[64:128, :n] = xE[b, :, i0:i0 + n]
        xs1c = np.zeros((128, TLOC), np.float16)
        xs1c[:, 0:TLOC - 1] = xbc[:, 1:TLOC]
        in_maps.append({"xb": xbc, "xb2": xb2c, "xs1": xs1c, "w2": w2,
                       "sid": sid, "wc": wc, "cst": cst})
    meta = dict(B=B, T=T, P=P, NSB=NSB, NTOT=NTOT)
    return in_maps, meta


def _assemble(results, meta, get):
    B, T, P, NSB = meta["B"], meta["T"], meta["P"], meta["NSB"]
    out = np.empty((B, T, 64), np.float32)
    for core in range(2 * B):
        b, h = core // 2, core % 2
        od = np.asarray(get(core), np.float32)  # [128, 512*NSB]
        arr = od.reshape(2, 64, NSB, 512)       # (par, d, sb, j)
        half = arr.transpose(2, 0, 3, 1).reshape(1024 * NSB, 64)[:P]
        out[b, h * P:(h + 1) * P, :] = half
    return out


def kernel(x, weights, w_conv, scale1, bias1, scale2, bias2, _sim=False):
    in_maps, meta = _prep_inputs(x, weights, w_conv, scale1, bias1, scale2,
                                 bias2)
    nc = _get_program(meta["NSB"], meta["NTOT"], meta["P"])

    if _sim:
        from concourse.bass_interp import MultiCoreSim
        sim = MultiCoreSim(nc, num_cores=8)
        for core in range(8):
            for name, arr in in_maps[core].items():
                sim.cores[core].tensor(name)[:] = arr
        sim.simulate(check_with_hw=False)
        return _assemble(results=None, meta=meta,
                         get=lambda c: sim.cores[c].tensor("out"))

    from concourse.bass_utils import run_bass_kernel_spmd
    res = run_bass_kernel_spmd(nc, in_maps, list(range(8)))
    return _assemble(results=None, meta=meta,
                     get=lambda c: res.results[c]["out"])


if __name__ == "__main__":
    pass


# revision 20
# speedup vs baseline: 13113.2590x; 13113.2590x over previous
"""Trainium2 Bass kernel for nn_ConvblockWithTarget (dense_cnn).

Reference computation (B=4, L=32768, C=64, K=7, T=16378):
  - unfold x into windows of 2K at stride 2 -> xp[b,c,t,j] = x[b, 2t+j, c]
  - dynamic gates: wfull[b,d,l,m] = sum_c x[b, 12+l, c] * weights[d,c,m]
    gate[b,d,t,j] = wfull[b,d,2t+(j%2), j//2];  g = tanh(gate)
  - y[b,t,d] = sum_j xp * g  + skip x[b, 12+2t, d]
  - y_ = batchnorm1(y);  z = y_ @ w_conv; z = gelu_tanh(batchnorm2(z)); out = y_ + z

Sharding: 8 cores = (batch b, sequence-half h).  Each core computes
P = T//2 output positions for one batch.  Batch-norm statistics are
global -> two tiny AllReduces of per-channel (sum, sumsq).

On-chip layout is channel-major ("transposed"): SBUF partitions hold
(channel d, phase/block parity).  Host-side prep uploads x pre-split
into even/odd phases, transposed and cast to fp16:
  XB  = [xE | xO] stacked on partitions (rows 0-63 = even-phase, 64-127 odd)
  XS1 = XB shifted by one element (keeps DVE 2x mode for odd window taps)

Per 1024-output superblock: 7 gate matmuls (row-tiled pairs, both phases
concurrently), tanh on the scalar engine straight out of PSUM, fp16
tensor-tensor products on the vector engine, and the 14-tap window sum done
back on the tensor engine by accumulating identity-matmuls into PSUM
(partition-halves summed by a stacked [I;I] stationary operand, skip
connection seeded via [I;0]).  Statistics use bn_stats/bn_aggr and a single
AllGather per batch-norm; rsqrt = ACT sqrt + DVE reciprocal + one Newton
step.  The 1x1 conv runs on raw y with the BN1 scale folded into its
weights and the BN1 shift folded into a per-channel constant handled in the
BN2/gelu affine.
"""

import os
import numpy as np

K = 7


# ----------------------------------------------------------------------------
# device program
# ----------------------------------------------------------------------------

def _build_program(NSB, NTOT, P, n_cores=8, repeat_a=1, no_cc=False, repeat_bc=1):
    """Emit the SPMD Bass program. NSB = superblocks (1024 outputs each) per
    core; NTOT = total valid samples across all cores (= B*T); P = valid
    outputs per core."""
    from contextlib import ExitStack

    import concourse.bacc as bacc
    import concourse.bass as bass
    import concourse.mybir as mybir
    import concourse.tile as tile
    from concourse.alu_op_type import AluOpType as alu

    dt = mybir.dt
    f32, f16 = dt.float32, dt.float16
    AF = mybir.ActivationFunctionType
    EPS = 1e-5
    TLOC = 1024 * NSB + 16
    FO = 512 * NSB  # output free size per partition

    nc = bacc.Bacc("TRN2", target_bir_lowering=False, debug=False,
                   num_devices=n_cores)

    xb_d = nc.dram_tensor("xb", [128, TLOC], f16, kind="ExternalInput")
    xs1_d = nc.dram_tensor("xs1", [128, TLOC], f16, kind="ExternalInput")
    w2_d = nc.dram_tensor("w2", [128, 64 * K], f16, kind="ExternalInput")
    sid_d = nc.dram_tensor("sid", [128, 128], f16, kind="ExternalInput")
    wc_d = nc.dram_tensor("wc", [128, 64], f16, kind="ExternalInput")
    cst_d = nc.dram_tensor("cst", [128, 8], f32, kind="ExternalInput")
    out_d = nc.dram_tensor("out", [128, FO], f16, kind="ExternalOutput")

    groups = [list(range(n_cores))]

    with ExitStack() as es:
        tc = es.enter_context(tile.TileContext(nc))
        cp = es.enter_context(tc.tile_pool(name="const", bufs=1))
        dp = es.enter_context(tc.tile_pool(name="dram", bufs=1, space="DRAM"))
        gsb = es.enter_context(tc.tile_pool(name="gsb", bufs=3))
        prsb = es.enter_context(tc.tile_pool(name="prsb", bufs=3))
        osb = es.enter_context(tc.tile_pool(name="osb", bufs=2))
        scp = es.enter_context(tc.tile_pool(name="scp", bufs=1))

        # persistent SBUF
        xb_sb = cp.tile([128, TLOC], f16)
        xs1_sb = cp.tile([128, TLOC], f16)
        w2_sb = cp.tile([128, 64 * K], f16)
        sid_sb = cp.tile([128, 128], f16)
        wc_sb = cp.tile([128, 64], f16)
        cst_sb = cp.tile([128, 8], f32)
        y_sb = cp.tile([128, FO], f16)
        yn_sb = cp.tile([128, FO], f16)
        yst = cp.tile([128, 6 * NSB], f32)
        zst = cp.tile([128, 6 * NSB], f32)

        nc.sync.dma_start(out=w2_sb[:], in_=w2_d.ap())
        nc.sync.dma_start(out=sid_sb[:], in_=sid_d.ap())
        nc.sync.dma_start(out=wc_sb[:], in_=wc_d.ap())
        nc.sync.dma_start(out=cst_sb[:], in_=cst_d.ap())
        # chunked x loads so superblock 0's compute starts immediately
        for sb in range(NSB + 1):
            lo = 1024 * sb
            hi = min(TLOC, lo + 1024)
            if lo >= hi:
                continue
            sl = slice(lo, hi)
            nc.sync.dma_start(out=xb_sb[:, sl], in_=xb_d.ap()[:, sl])
            nc.sync.dma_start(out=xs1_sb[:, sl], in_=xs1_d.ap()[:, sl])

        SI = sid_sb[:, 0:64]     # [I64; I64]
        SIE = sid_sb[:, 64:128]  # [I64; 0]

        def mm(out, lhsT, rhs, tp, start=True, stop=True):
            nc.tensor.matmul(out, lhsT, rhs, start=start, stop=stop,
                             tile_position=tp, skip_group_check=True)

        def allreduce_2(arv, tagn):
            """AllReduce a [128,2] f32 (sum, sumsq); returns [128,2] totals
            combined across cores AND across the two partition halves
            (channel d and d+64 hold the same channel)."""
            arin = dp.tile([128, 2], f32, name=f"arin{tagn}")
            arout = dp.tile([128, 2], f32, name=f"arout{tagn}",
                            addr_space="Shared")
            nc.sync.dma_start(out=arin[:], in_=arv[:])
            R = n_cores
            agout = dp.tile([128 * R, 2], f32, name=f"agout{tagn}",
                            addr_space="Shared")
            if n_cores == 1 or no_cc:
                # single-core variant (TimelineSim profiling) or timing
                # ablation: skip the collective (numerically wrong)
                nc.sync.dma_start(out=agout[0:128, :], in_=arin[:])
            else:
                nc.gpsimd.collective_compute(
                    "AllGather", alu.bypass, replica_groups=groups,
                    ins=[arin.opt()], outs=[agout.opt()])
            # gather all (rank, parity) partials for channel d onto
            # partition d of both halves, then reduce
            gat = scp.tile([128, 2 * 2 * R], f32, name=f"gat{tagn}")
            src = agout.rearrange("(r a d) s -> d s (r a)", r=R, a=2)
            nc.sync.dma_start(
                out=gat[0:64, :].rearrange("d (s ra) -> d s ra", s=2),
                in_=src)
            nc.sync.dma_start(
                out=gat[64:128, :].rearrange("d (s ra) -> d s ra", s=2),
                in_=src)
            tot = scp.tile([128, 2], f32, name=f"tot{tagn}")
            nc.vector.tensor_reduce(
                tot[:], gat[:].rearrange("p (s ra) -> p s ra", s=2),
                mybir.AxisListType.X, alu.add)
            return tot

        def rsqrt_of(var_ap, tagn):
            """[128,1] rsqrt(var + EPS) via ACT sqrt + DVE recip + one Newton
            step (r1 = r0*(1.5 - 0.5*v*r0^2))."""
            vp = scp.tile([128, 1], f32, name=f"vp{tagn}")
            nc.vector.tensor_scalar(vp[:], var_ap, EPS, None, alu.add)
            sq = scp.tile([128, 1], f32, name=f"sq{tagn}")
            nc.scalar.sqrt(sq[:], vp[:])
            r0 = scp.tile([128, 1], f32, name=f"r0{tagn}")
            nc.vector.reciprocal(r0[:], sq[:])
            t2 = scp.tile([128, 1], f32, name=f"t2{tagn}")
            nc.vector.tensor_tensor(t2[:], r0[:], r0[:], alu.mult)
            nc.vector.tensor_tensor(t2[:], t2[:], vp[:], alu.mult)
            nc.vector.tensor_scalar(t2[:], t2[:], -0.5, 1.5, alu.mult, alu.add)
            rs = scp.tile([128, 1], f32, name=f"rs{tagn}")
            nc.vector.tensor_tensor(rs[:], r0[:], t2[:], alu.mult)
            return rs

        def sums_from_bnstats(stt, tagn):
            """bn_stats slots [128, 6*NSB] -> [128,2] (sum, sumsq) local."""
            agg = scp.tile([128, 2], f32, name=f"agg{tagn}")
            nc.vector.bn_aggr(agg[:], stt[:])
            cnt = float(512 * NSB)
            arv = scp.tile([128, 2], f32, name=f"arv{tagn}")
            nc.vector.tensor_scalar(arv[:, 0:1], agg[:, 0:1], cnt, None,
                                    alu.mult)
            m2 = scp.tile([128, 1], f32, name=f"m2{tagn}")
            nc.vector.tensor_tensor(m2[:], agg[:, 0:1], agg[:, 0:1], alu.mult)
            nc.vector.tensor_tensor(m2[:], m2[:], agg[:, 1:2], alu.add)
            nc.vector.tensor_scalar(arv[:, 1:2], m2[:], cnt, None, alu.mult)
            return arv

        # ------------------------------------------------------------------
        # Phase A: gates -> tanh -> windowed products -> PE-summed y
        # ------------------------------------------------------------------
        with tc.tile_pool(name="psA", bufs=2, space="PSUM") as psA:
            rep_cm = tc.For_i(0, repeat_a, 1) if repeat_a > 1 else None
            if rep_cm is not None:
                rep_cm.__enter__()
            for sb in range(NSB):
                b6 = 1024 * sb + 6
                y_ps = psA.tile([128, 512], f32, tag="y_ps", name=f"y_ps{sb}")
                # skip connection seeds the accumulation (lhsT [I64;0])
                mm(y_ps[0:64, :], SIE, xb_sb[:, b6:b6 + 512], (0, 0),
                   start=True, stop=False)
                mm(y_ps[64:128, :], SIE, xb_sb[:, b6 + 512:b6 + 1024], (0, 64),
                   start=True, stop=False)
                for m in range(K):
                    wsl = slice(64 * m, 64 * m + 64)
                    g_ps = psA.tile([128, 1024], f32, tag="g_ps",
                                    name=f"g_ps{sb}_{m}")
                    # row-tiled matmul pairs (rows=contraction c); N=512 per
                    # MM (PSUM bank limit), two rounds cover both i-blocks
                    mm(g_ps[0:64, 0:512], w2_sb[0:64, wsl],
                       xb_sb[0:64, b6:b6 + 512], (0, 0))
                    mm(g_ps[64:128, 0:512], w2_sb[64:128, wsl],
                       xb_sb[64:128, b6:b6 + 512], (64, 64))
                    mm(g_ps[0:64, 512:1024], w2_sb[0:64, wsl],
                       xb_sb[0:64, b6 + 512:b6 + 1024], (0, 0))
                    mm(g_ps[64:128, 512:1024], w2_sb[64:128, wsl],
                       xb_sb[64:128, b6 + 512:b6 + 1024], (64, 64))
                    gs = gsb.tile([128, 1024], f16, tag="gs",
                                  name=f"gs{sb}_{m}")
                    nc.scalar.activation(gs[:], g_ps[:], AF.Tanh)
                    pr = prsb.tile([128, 1024], f16, tag="pr",
                                   name=f"pr{sb}_{m}")
                    if m % 2 == 0:
                        xop = xb_sb[:, 1024 * sb + m:1024 * sb + m + 1024]
                    else:
                        xop = xs1_sb[:, 1024 * sb + m - 1:
                                     1024 * sb + m - 1 + 1024]
                    nc.vector.tensor_tensor(pr[:], gs[:], xop, alu.mult)
                    mm(y_ps[0:64, :], SI, pr[:, 0:512], (0, 0),
                       start=False, stop=(m == K - 1))
                    mm(y_ps[64:128, :], SI, pr[:, 512:1024], (0, 64),
                       start=False, stop=(m == K - 1))
                # zero positions beyond this core's valid range (i >= P):
                # they hold the neighbouring half's data and would corrupt
                # the global statistics.
                jA = max(0, min(512, P - 1024 * sb))
                jB = max(0, min(512, P - 1024 * sb - 512))
                if jA < 512:
                    nc.vector.tensor_scalar(y_ps[0:64, jA:512],
                                            y_ps[0:64, jA:512], 0.0, None,
                                            alu.mult)
                if jB < 512:
                    nc.vector.tensor_scalar(y_ps[64:128, jB:512],
                                            y_ps[64:128, jB:512], 0.0, None,
                                            alu.mult)
                nc.vector.bn_stats(yst[:, 6 * sb:6 * sb + 6], y_ps[:])
                nc.vector.tensor_copy(y_sb[:, 512 * sb:512 * sb + 512],
                                      y_ps[:])
            if rep_cm is not None:
                rep_cm.__exit__(None, None, None)

            # ---- AllReduce 1: y statistics ----
            arv1 = sums_from_bnstats(yst, 1)
            tot1 = allreduce_2(arv1, 1)

            mean1 = scp.tile([128, 1], f32)
            nc.vector.tensor_scalar(mean1[:], tot1[:, 0:1], 1.0 / NTOT, None,
                                    alu.mult)
            var1 = scp.tile([128, 1], f32)
            nc.vector.tensor_scalar(var1[:], tot1[:, 1:2], 1.0 / NTOT, None,
                                    alu.mult)
            msq = scp.tile([128, 1], f32)
            nc.vector.tensor_tensor(msq[:], mean1[:], mean1[:], alu.mult)
            nc.vector.tensor_sub(var1[:], var1[:], msq[:])
            rstd1 = rsqrt_of(var1[:], 1)
            s1h = scp.tile([128, 1], f32)
            nc.vector.tensor_tensor(s1h[:], rstd1[:], cst_sb[:, 0:1], alu.mult)
            t1 = scp.tile([128, 1], f32)
            nc.vector.tensor_tensor(t1[:], mean1[:], s1h[:], alu.mult)
            nc.vector.tensor_sub(t1[:], cst_sb[:, 1:2], t1[:])
            # conv weights pre-scaled by s1h (per input channel = partition)
            wcs = scp.tile([128, 64], f16)
            nc.vector.tensor_scalar(wcs[:], wc_sb[:], s1h[:], None, alu.mult)
            t1h = scp.tile([128, 1], f16)
            nc.vector.tensor_copy(t1h[:], t1[:])
            # c_row[d'] = sum_d t1[d] * wc[d,d']  (borrow a y_ps slot)
            c_ps = psA.tile([128, 1], f32, tag="y_ps")
            mm(c_ps[0:64, :], wc_sb[0:64, :], t1h[0:64, :], (0, 0))
            mm(c_ps[64:128, :], wc_sb[64:128, :], t1h[64:128, :], (64, 64))
            c_row = scp.tile([128, 1], f32)
            nc.vector.tensor_copy(c_row[:], c_ps[:])

        # ------------------------------------------------------------------
        # Phase B: 1x1 conv (on raw y; scale folded in weights) + z stats
        # ------------------------------------------------------------------
        with tc.tile_pool(name="psZ", bufs=NSB, space="PSUM") as psZ:
            repb_cm = tc.For_i(0, repeat_bc, 1) if repeat_bc > 1 else None
            if repb_cm is not None:
                repb_cm.__enter__()
            z_tiles = []
            for sb in range(NSB):
                osl = slice(512 * sb, 512 * sb + 512)
                z_ps = psZ.tile([128, 512], f32, tag="z_ps", name=f"z_ps{sb}")
                mm(z_ps[0:64, :], wcs[0:64, :], y_sb[0:64, osl], (0, 0))
                mm(z_ps[64:128, :], wcs[64:128, :], y_sb[64:128, osl],
                   (64, 64))
                nc.vector.bn_stats(zst[:, 6 * sb:6 * sb + 6], z_ps[:])
                z_tiles.append(z_ps)

            # BN1 application (overlaps collective 2)
            for sb in range(NSB):
                osl = slice(512 * sb, 512 * sb + 512)
                nc.vector.tensor_scalar(yn_sb[:, osl], y_sb[:, osl], s1h[:],
                                        t1[:], alu.mult, alu.add)
            if repb_cm is not None:
                repb_cm.__exit__(None, None, None)

            # ---- AllReduce 2: z statistics (of z_mm; z = z_mm + c_row) ----
            arv2 = sums_from_bnstats(zst, 2)
            tot2 = allreduce_2(arv2, 2)

            repc_cm = tc.For_i(0, repeat_bc, 1) if repeat_bc > 1 else None
            if repc_cm is not None:
                repc_cm.__enter__()
            szm = tot2[:, 0:1]   # sum z_mm
            sz2m = tot2[:, 1:2]  # sum z_mm^2
            ncr = scp.tile([128, 1], f32)
            nc.vector.tensor_scalar(ncr[:], c_row[:], float(NTOT), None,
                                    alu.mult)
            sz = scp.tile([128, 1], f32)
            nc.vector.tensor_tensor(sz[:], szm, ncr[:], alu.add)
            # sumsq z = sumsq z_mm + 2*c_row*sum z_mm + NTOT*c_row^2
            a1 = scp.tile([128, 1], f32)
            nc.vector.tensor_tensor(a1[:], c_row[:], szm, alu.mult)
            nc.vector.tensor_scalar(a1[:], a1[:], 2.0, None, alu.mult)
            a2 = scp.tile([128, 1], f32)
            nc.vector.tensor_tensor(a2[:], c_row[:], ncr[:], alu.mult)
            sz2 = scp.tile([128, 1], f32)
            nc.vector.tensor_tensor(sz2[:], sz2m, a1[:], alu.add)
            nc.vector.tensor_tensor(sz2[:], sz2[:], a2[:], alu.add)
            mean2 = scp.tile([128, 1], f32)
            nc.vector.tensor_scalar(mean2[:], sz[:], 1.0 / NTOT, None,
                                    alu.mult)
            var2 = scp.tile([128, 1], f32)
            nc.vector.tensor_scalar(var2[:], sz2[:], 1.0 / NTOT, None,
                                    alu.mult)
            msq2 = scp.tile([128, 1], f32)
            nc.vector.tensor_tensor(msq2[:], mean2[:], mean2[:], alu.mult)
            nc.vector.tensor_sub(var2[:], var2[:], msq2[:])
            rstd2 = rsqrt_of(var2[:], 2)
            s2h = scp.tile([128, 1], f32)
            nc.vector.tensor_tensor(s2h[:], rstd2[:], cst_sb[:, 2:3], alu.mult)
            # gelu input = z_mm*s2h + gb, gb = b2 + s2h*(c_row - mean2)
            gb = scp.tile([128, 1], f32)
            nc.vector.tensor_sub(gb[:], c_row[:], mean2[:])
            nc.vector.tensor_tensor(gb[:], gb[:], s2h[:], alu.mult)
            nc.vector.tensor_tensor(gb[:], gb[:], cst_sb[:, 3:4], alu.add)

            # --------------------------------------------------------------
            # Phase C: gelu + residual + store
            # --------------------------------------------------------------
            for sb in range(NSB):
                osl = slice(512 * sb, 512 * sb + 512)
                ge = gsb.tile([128, 512], f16, tag="ge", name=f"ge{sb}")
                nc.scalar.activation(ge[:], z_tiles[sb][:],
                                     AF.Gelu_apprx_tanh, bias=gb[:],
                                     scale=s2h[:])
                of = osb.tile([128, 512], f16, tag="of", name=f"of{sb}")
                nc.vector.tensor_tensor(of[:], ge[:], yn_sb[:, osl], alu.add)
                nc.sync.dma_start(out=out_d.ap()[:, osl], in_=of[:])
            if repc_cm is not None:
                repc_cm.__exit__(None, None, None)

    nc.compile()
    return nc


# ----------------------------------------------------------------------------
# host side
# ----------------------------------------------------------------------------

_CACHE = {}


def _get_program(NSB, NTOT, P):
    key = (NSB, NTOT, P)
    if key not in _CACHE:
        _CACHE[key] = _build_program(NSB, NTOT, P)
    return _CACHE[key]


def _prep_inputs(x, weights, w_conv, scale1, bias1, scale2, bias2):
    """Host-side layout prep. Returns (in_maps, meta)."""
    B, L, C = x.shape
    T = (L - 2 * K) // 2 + 1
    assert T % 2 == 0
    P = T // 2
    NSB = -(-P // 1024)
    TLOC = 1024 * NSB + 16
    NTOT = B * T
    LH = L // 2

    x = np.asarray(x, np.float32)
    xE = np.ascontiguousarray(x[:, 0::2, :].transpose(0, 2, 1)).astype(np.float16)
    xO = np.ascontiguousarray(x[:, 1::2, :].transpose(0, 2, 1)).astype(np.float16)

    w2 = np.zeros((128, 64 * K), np.float16)
    wt = np.asarray(weights, np.float32)  # (C,C,K) = (d,c,m)
    for m in range(K):
        w2[0:64, 64 * m:64 * m + 64] = wt[:, :, m].T.astype(np.float16)
    w2[64:128] = w2[0:64]

    sid = np.zeros((128, 128), np.float16)
    eye = np.eye(64, dtype=np.float16)
    sid[0:64, 0:64] = eye
    sid[64:128, 0:64] = eye
    sid[0:64, 64:128] = eye

    wc = np.zeros((128, 64), np.float16)
    wc[0:64] = np.asarray(w_conv, np.float16)
    wc[64:128] = wc[0:64]

    cst = np.zeros((128, 8), np.float32)
    for i, v in enumerate([scale1, bias1, scale2, bias2]):
        v = np.asarray(v, np.float32)
        cst[0:64, i] = v
        cst[64:128, i] = v

    in_maps = []
    for core in range(2 * B):
        b, h = core // 2, core % 2
        i0 = h * P
        n = min(TLOC, LH - i0)
        xbc = np.zeros((128, TLOC), np.float16)
        xbc[0:64, :n] = xE[b, :, i0:i0 + n]
        xbc[64:128, :n] = xO[b, :, i0:i0 + n]
        xs1c = np.zeros((128, TLOC), np.float16)
        xs1c[:, 0:TLOC - 1] = xbc[:, 1:TLOC]
        in_maps.append({"xb": xbc, "xs1": xs1c, "w2": w2,
                       "sid": sid, "wc": wc, "cst": cst})
    meta = dict(B=B, T=T, P=P, NSB=NSB, NTOT=NTOT)
    return in_maps, meta


def _assemble(results, meta, get):
    B, T, P, NSB = meta["B"], meta["T"], meta["P"], meta["NSB"]
    out = np.empty((B, T, 64), np.float32)
    for core in range(2 * B):
        b, h = core // 2, core % 2
        od = np.asarray(get(core), np.float32)  # [128, 512*NSB]
        arr = od.reshape(2, 64, NSB, 512)       # (par, d, sb, j)
        half = arr.transpose(2, 0, 3, 1).reshape(1024 * NSB, 64)[:P]
        out[b, h * P:(h + 1) * P, :] = half
    return out


def kernel(x, weights, w_conv, scale1, bias1, scale2, bias2, _sim=False):
    in_maps, meta = _prep_inputs(x, weights, w_conv, scale1, bias1, scale2,
                                 bias2)
    nc = _get_program(meta["NSB"], meta["NTOT"], meta["P"])

    if _sim:
        from concourse.bass_interp import MultiCoreSim
        sim = MultiCoreSim(nc, num_cores=8)
        for core in range(8):
            for name, arr in in_maps[core].items():
                sim.cores[core].tensor(name)[:] = arr
        sim.simulate(check_with_hw=False)
        return _assemble(results=None, meta=meta,
                         get=lambda c: sim.cores[c].tensor("out"))

    from concourse.bass_utils import run_bass_kernel_spmd
    res = run_bass_kernel_spmd(nc, in_maps, list(range(8)))
    return _assemble(results=None, meta=meta,
                     get=lambda c: res.results[c]["out"])


if __name__ == "__main__":
    pass
